# revision 16
# baseline (speedup 1.0000x reference)
"""Trainium2 Bass kernel for AnchorHead: three fused 1x1-conv heads.

Computes cls/reg/obj = x @ W_* + b_* for x [400000, 128] by sharding rows
across 8 NeuronCores.

Per-core layout tricks:
- Input rows are loaded "grouped": partition p holds G consecutive DRAM rows,
  so every input DMA line is G*512B contiguous (full HBM efficiency). The
  resulting column permutation commutes with transpose+GEMM and is undone
  for free by host-side reindexing while unsharding.
- x^T is built on-chip with PE transpose-mode matmuls (exact, fp32).
- The GEMM runs with W [128, 108] stationary and x^T moving (N=512) in
  float32r (single-pass, 4x the fp32 matmul rate; inputs pre-rounded).
- out^T [108, rows] stays transposed in DRAM so output DMA lines are 4KB+
  contiguous; the host re-transposes while unsharding.
"""

import numpy as np

import concourse.bass as bass
import concourse.tile as tile
from concourse import bacc, masks, mybir
from concourse.bass_utils import run_bass_kernel_spmd

N_CORES = 8
N_VOX = 400000
IN_FEAT = 128
N_OUT = 108  # 60 cls + 42 reg + 6 obj
ROWS_PER_CORE = N_VOX // N_CORES  # 50000
ROWS_PAD = 50176  # 24*2048 + 1024
SUPERS = [2048] * 24 + [1024]  # per-core DMA batches; G = s // 128 rows/partition
PAIR = 1024  # columns per PSUM tile pair (2 banks)
GEMM_F32R = True


def build_nc(supers=None):
    supers = supers or SUPERS
    rows_pad = sum(supers)
    nc = bacc.Bacc("TRN2", target_bir_lowering=False, debug=False,
                   num_devices=N_CORES)
    x = nc.dram_tensor("x", [rows_pad, IN_FEAT], mybir.dt.float32,
                       kind="ExternalInput").ap()
    w = nc.dram_tensor("w", [IN_FEAT, N_OUT], mybir.dt.float32,
                       kind="ExternalInput").ap()
    b = nc.dram_tensor("b", [N_OUT, 1], mybir.dt.float32,
                       kind="ExternalInput").ap()
    out = nc.dram_tensor("out", [N_OUT, rows_pad], mybir.dt.float32,
                         kind="ExternalOutput").ap()

    gemm_dt = mybir.dt.float32r if GEMM_F32R else mybir.dt.float32

    with tile.TileContext(nc) as tc:
        with (
            tc.tile_pool(name="const", bufs=1) as const_pool,
            tc.tile_pool(name="xnat", bufs=6) as xnat_pool,
            tc.tile_pool(name="xt", bufs=6) as xt_pool,
            tc.tile_pool(name="outsb", bufs=4) as out_pool,
            tc.tile_pool(name="xtp", bufs=2, space="PSUM") as xtp_pool,
            tc.tile_pool(name="outp", bufs=2, space="PSUM") as outp_pool,
        ):
            ident = const_pool.tile([128, 128], mybir.dt.float32)
            masks.make_identity(nc, ident[:])
            w_sb = const_pool.tile([IN_FEAT, N_OUT], mybir.dt.float32)
            nc.sync.dma_start(w_sb[:], w[:, :])
            b_sb = const_pool.tile([N_OUT, 1], mybir.dt.float32)
            nc.sync.dma_start(b_sb[:], b[:, :])
            w_r = const_pool.tile([IN_FEAT, N_OUT], gemm_dt)
            nc.vector.tensor_copy(w_r[:], w_sb[:])

            row0 = 0
            for s_rows in supers:
                g_tot = s_rows // 128  # row-groups in this super
                xnat = xnat_pool.tile([128, s_rows], mybir.dt.float32)
                xnat3 = xnat[:].rearrange("p (g d) -> p g d", d=IN_FEAT)
                # DRAM rows row0 + p*g_tot + g  ->  partition p, group g
                x_sup = x[row0:row0 + s_rows, :].rearrange(
                    "(p g) d -> p g d", g=g_tot)
                nc.sync.dma_start(xnat3, x_sup)

                out_sb = out_pool.tile([128, s_rows], mybir.dt.float32)
                for c0 in range(0, s_rows, PAIR):
                    cols = min(PAIR, s_rows - c0)
                    xtp = xtp_pool.tile([128, PAIR], mybir.dt.float32)
                    for i in range(cols // 128):
                        g = (c0 // 128) + i
                        nc.tensor.matmul(
                            xtp[:, i * 128:(i + 1) * 128],
                            xnat3[:, g, :],
                            ident[:],
                            is_transpose=True,
                        )
                    xts = xt_pool.tile([128, PAIR], gemm_dt)
                    nc.scalar.copy(xts[:, :cols], xtp[:, :cols])

                    outp = outp_pool.tile([128, PAIR], mybir.dt.float32)
                    for j in range(0, cols, 512):
                        nc.tensor.matmul(outp[:N_OUT, j:j + 512],
                                         w_r[:], xts[:, j:j + 512])
                    nc.vector.tensor_scalar_add(
                        out_sb[:N_OUT, c0:c0 + cols],
                        outp[:N_OUT, :cols],
                        b_sb[:, :],
                    )
                    # out-DMAs avoid the SP ring (whose data dependency
                    # would head-of-line block input DMAs); alternating the
                    # gpsimd/scalar rings doubles issue parallelism
                    nc.gpsimd.dma_start(out[:, row0 + c0:row0 + c0 + cols],
                                        out_sb[:N_OUT, c0:c0 + cols])

                row0 += s_rows

    nc.compile()
    return nc


_NC_CACHE = {}


def _get_nc():
    if "nc" not in _NC_CACHE:
        _NC_CACHE["nc"] = build_nc()
    return _NC_CACHE["nc"]


def _unpermute_cols(out_t, supers):
    """Invert the grouped-row loading permutation, per super-batch."""
    parts = []
    col0 = 0
    for s_rows in supers:
        g_tot = s_rows // 128
        blk = out_t[:, col0:col0 + s_rows]
        # column g*128 + p  ->  row p*g_tot + g
        parts.append(blk.reshape(N_OUT, g_tot, 128).transpose(0, 2, 1)
                     .reshape(N_OUT, s_rows))
        col0 += s_rows
    return np.concatenate(parts, axis=1)


def kernel(x, W_cls, b_cls, W_obj, b_obj, W_reg, b_reg):
    x = np.asarray(x, dtype=np.float32)
    w_all = np.concatenate(
        [np.asarray(W_cls, np.float32), np.asarray(W_reg, np.float32),
         np.asarray(W_obj, np.float32)], axis=1)
    b_all = np.concatenate(
        [np.asarray(b_cls, np.float32), np.asarray(b_reg, np.float32),
         np.asarray(b_obj, np.float32)]).reshape(N_OUT, 1)

    x_sh = x.reshape(N_CORES, ROWS_PER_CORE, IN_FEAT)
    x_pad = np.zeros((N_CORES, ROWS_PAD, IN_FEAT), dtype=np.float32)
    x_pad[:, :ROWS_PER_CORE] = x_sh

    in_maps = [{"x": x_pad[i], "w": w_all, "b": b_all} for i in range(N_CORES)]

    nc = _get_nc()
    res = run_bass_kernel_spmd(nc, in_maps, core_ids=list(range(N_CORES)))

    out_t = np.concatenate(
        [_unpermute_cols(res.results[i]["out"], SUPERS)[:, :ROWS_PER_CORE]
         for i in range(N_CORES)],
        axis=1)  # [108, 400000]
    out = out_t.T  # view
    cls_pred = np.ascontiguousarray(out[:, :60])
    box_reg = np.ascontiguousarray(out[:, 60:102])
    objness = np.ascontiguousarray(out[:, 102:108])
    return cls_pred, box_reg, objness


# revision 17
# speedup vs baseline: 1.0210x; 1.0210x over previous
"""Trainium2 Bass kernel for AnchorHead: three fused 1x1-conv heads.

Computes cls/reg/obj = x @ W_* + b_* for x [400000, 128] by sharding rows
across 8 NeuronCores.

Per-core layout tricks:
- Input rows are loaded "grouped": partition p holds G consecutive DRAM rows,
  so every input DMA line is G*512B contiguous (full HBM efficiency). The
  resulting column permutation commutes with transpose+GEMM and is undone
  for free by host-side reindexing while unsharding.
- x^T is built on-chip with PE transpose-mode matmuls (exact, fp32).
- The GEMM runs with W [128, 108] stationary and x^T moving (N=512) in
  float32r (single-pass, 4x the fp32 matmul rate; inputs pre-rounded).
- out^T [108, rows] stays transposed in DRAM so output DMA lines are 4KB+
  contiguous; the host re-transposes while unsharding.
"""

import numpy as np

import concourse.bass as bass
import concourse.tile as tile
from concourse import bacc, masks, mybir
from concourse.bass_utils import run_bass_kernel_spmd

N_CORES = 8
N_VOX = 400000
IN_FEAT = 128
N_OUT = 108  # 60 cls + 42 reg + 6 obj
ROWS_PER_CORE = N_VOX // N_CORES  # 50000
ROWS_PAD = 50176  # graduated head + 23*2048 + 1024
SUPERS = [512, 512, 1024] + [2048] * 23 + [1024]  # per-core DMA batches
PAIR = 1024  # columns per PSUM tile pair (2 banks)
GEMM_F32R = True


def build_nc(supers=None):
    supers = supers or SUPERS
    rows_pad = sum(supers)
    nc = bacc.Bacc("TRN2", target_bir_lowering=False, debug=False,
                   num_devices=N_CORES)
    x = nc.dram_tensor("x", [rows_pad, IN_FEAT], mybir.dt.float32,
                       kind="ExternalInput").ap()
    w = nc.dram_tensor("w", [IN_FEAT, N_OUT], mybir.dt.float32,
                       kind="ExternalInput").ap()
    b = nc.dram_tensor("b", [N_OUT, 1], mybir.dt.float32,
                       kind="ExternalInput").ap()
    out = nc.dram_tensor("out", [N_OUT, rows_pad], mybir.dt.float32,
                         kind="ExternalOutput").ap()

    gemm_dt = mybir.dt.float32r if GEMM_F32R else mybir.dt.float32

    with tile.TileContext(nc) as tc:
        with (
            tc.tile_pool(name="const", bufs=1) as const_pool,
            tc.tile_pool(name="xnat", bufs=6) as xnat_pool,
            tc.tile_pool(name="xt", bufs=6) as xt_pool,
            tc.tile_pool(name="outsb", bufs=4) as out_pool,
            tc.tile_pool(name="xtp", bufs=2, space="PSUM") as xtp_pool,
            tc.tile_pool(name="outp", bufs=2, space="PSUM") as outp_pool,
        ):
            ident = const_pool.tile([128, 128], mybir.dt.float32)
            masks.make_identity(nc, ident[:])
            w_sb = const_pool.tile([IN_FEAT, N_OUT], mybir.dt.float32)
            nc.sync.dma_start(w_sb[:], w[:, :])
            b_sb = const_pool.tile([N_OUT, 1], mybir.dt.float32)
            nc.sync.dma_start(b_sb[:], b[:, :])
            w_r = const_pool.tile([IN_FEAT, N_OUT], gemm_dt)
            nc.vector.tensor_copy(w_r[:], w_sb[:])

            row0 = 0
            for s_rows in supers:
                g_tot = s_rows // 128  # row-groups in this super
                xnat = xnat_pool.tile([128, s_rows], mybir.dt.float32)
                xnat3 = xnat[:].rearrange("p (g d) -> p g d", d=IN_FEAT)
                # DRAM rows row0 + p*g_tot + g  ->  partition p, group g
                x_sup = x[row0:row0 + s_rows, :].rearrange(
                    "(p g) d -> p g d", g=g_tot)
                nc.sync.dma_start(xnat3, x_sup)

                out_sb = out_pool.tile([128, s_rows], mybir.dt.float32)
                for c0 in range(0, s_rows, PAIR):
                    cols = min(PAIR, s_rows - c0)
                    xtp = xtp_pool.tile([128, PAIR], mybir.dt.float32)
                    for i in range(cols // 128):
                        g = (c0 // 128) + i
                        nc.tensor.matmul(
                            xtp[:, i * 128:(i + 1) * 128],
                            xnat3[:, g, :],
                            ident[:],
                            is_transpose=True,
                        )
                    xts = xt_pool.tile([128, PAIR], gemm_dt)
                    nc.scalar.copy(xts[:, :cols], xtp[:, :cols])

                    outp = outp_pool.tile([128, PAIR], mybir.dt.float32)
                    for j in range(0, cols, 512):
                        nc.tensor.matmul(outp[:N_OUT, j:j + 512],
                                         w_r[:], xts[:, j:j + 512])
                    nc.vector.tensor_scalar_add(
                        out_sb[:N_OUT, c0:c0 + cols],
                        outp[:N_OUT, :cols],
                        b_sb[:, :],
                    )
                    # out-DMAs avoid the SP ring (whose data dependency
                    # would head-of-line block input DMAs); alternating the
                    # gpsimd/scalar rings doubles issue parallelism
                    if row0 >= rows_pad * 3 // 4:
                        rot = [nc.gpsimd, nc.scalar, nc.sync]
                    else:
                        rot = [nc.gpsimd, nc.scalar]
                    out_eng = rot[(row0 + c0) // PAIR % len(rot)]
                    out_eng.dma_start(out[:, row0 + c0:row0 + c0 + cols],
                                      out_sb[:N_OUT, c0:c0 + cols])

                row0 += s_rows

    nc.compile()
    return nc


_NC_CACHE = {}


def _get_nc():
    if "nc" not in _NC_CACHE:
        _NC_CACHE["nc"] = build_nc()
    return _NC_CACHE["nc"]


def _unpermute_cols(out_t, supers):
    """Invert the grouped-row loading permutation, per super-batch."""
    parts = []
    col0 = 0
    for s_rows in supers:
        g_tot = s_rows // 128
        blk = out_t[:, col0:col0 + s_rows]
        # column g*128 + p  ->  row p*g_tot + g
        parts.append(blk.reshape(N_OUT, g_tot, 128).transpose(0, 2, 1)
                     .reshape(N_OUT, s_rows))
        col0 += s_rows
    return np.concatenate(parts, axis=1)


def kernel(x, W_cls, b_cls, W_obj, b_obj, W_reg, b_reg):
    x = np.asarray(x, dtype=np.float32)
    w_all = np.concatenate(
        [np.asarray(W_cls, np.float32), np.asarray(W_reg, np.float32),
         np.asarray(W_obj, np.float32)], axis=1)
    b_all = np.concatenate(
        [np.asarray(b_cls, np.float32), np.asarray(b_reg, np.float32),
         np.asarray(b_obj, np.float32)]).reshape(N_OUT, 1)

    x_sh = x.reshape(N_CORES, ROWS_PER_CORE, IN_FEAT)
    x_pad = np.zeros((N_CORES, ROWS_PAD, IN_FEAT), dtype=np.float32)
    x_pad[:, :ROWS_PER_CORE] = x_sh

    in_maps = [{"x": x_pad[i], "w": w_all, "b": b_all} for i in range(N_CORES)]

    nc = _get_nc()
    res = run_bass_kernel_spmd(nc, in_maps, core_ids=list(range(N_CORES)))

    out_t = np.concatenate(
        [_unpermute_cols(res.results[i]["out"], SUPERS)[:, :ROWS_PER_CORE]
         for i in range(N_CORES)],
        axis=1)  # [108, 400000]
    out = out_t.T  # view
    cls_pred = np.ascontiguousarray(out[:, :60])
    box_reg = np.ascontiguousarray(out[:, 60:102])
    objness = np.ascontiguousarray(out[:, 102:108])
    return cls_pred, box_reg, objness


# revision 18
# speedup vs baseline: 1.0390x; 1.0176x over previous
"""Trainium2 Bass kernel for AnchorHead: three fused 1x1-conv heads.

Computes cls/reg/obj = x @ W_* + b_* for x [400000, 128] by sharding rows
across 8 NeuronCores.

Per-core layout tricks:
- Input rows are loaded "grouped": partition p holds G consecutive DRAM rows,
  so every input DMA line is G*512B contiguous (full HBM efficiency). The
  resulting column permutation commutes with transpose+GEMM and is undone
  for free by host-side reindexing while unsharding.
- x^T is built on-chip with PE transpose-mode matmuls (exact, fp32).
- The GEMM runs with W [128, 108] stationary and x^T moving (N=512) in
  float32r (single-pass, 4x the fp32 matmul rate; inputs pre-rounded).
- out^T [108, rows] stays transposed in DRAM so output DMA lines are 4KB+
  contiguous; the host re-transposes while unsharding.
"""

import numpy as np

import concourse.bass as bass
import concourse.tile as tile
from concourse import bacc, masks, mybir
from concourse.bass_utils import run_bass_kernel_spmd

N_CORES = 8
N_VOX = 400000
IN_FEAT = 128
N_OUT = 108  # 60 cls + 42 reg + 6 obj
ROWS_PER_CORE = N_VOX // N_CORES  # 50000
ROWS_PAD = 50176  # graduated head + 23*2048 + 1024
SUPERS = [512, 512, 1024] + [2048] * 23 + [1024]  # per-core DMA batches
PAIR = 1024  # columns per PSUM tile pair (2 banks)
GEMM_F32R = True


def build_nc(supers=None):
    supers = supers or SUPERS
    rows_pad = sum(supers)
    nc = bacc.Bacc("TRN2", target_bir_lowering=False, debug=False,
                   num_devices=N_CORES)
    x = nc.dram_tensor("x", [rows_pad, IN_FEAT], mybir.dt.float32,
                       kind="ExternalInput").ap()
    w = nc.dram_tensor("w", [IN_FEAT, N_OUT], mybir.dt.float32,
                       kind="ExternalInput").ap()
    b = nc.dram_tensor("b", [N_OUT, 1], mybir.dt.float32,
                       kind="ExternalInput").ap()
    out = nc.dram_tensor("out", [N_OUT, rows_pad], mybir.dt.float32,
                         kind="ExternalOutput").ap()

    gemm_dt = mybir.dt.float32r if GEMM_F32R else mybir.dt.float32

    with tile.TileContext(nc) as tc:
        with (
            tc.tile_pool(name="const", bufs=1) as const_pool,
            tc.tile_pool(name="xnat", bufs=6) as xnat_pool,
            tc.tile_pool(name="xt", bufs=6) as xt_pool,
            tc.tile_pool(name="outsb", bufs=4) as out_pool,
            tc.tile_pool(name="xtp", bufs=2, space="PSUM") as xtp_pool,
            tc.tile_pool(name="outp", bufs=2, space="PSUM") as outp_pool,
        ):
            ident = const_pool.tile([128, 128], mybir.dt.float32)
            masks.make_identity(nc, ident[:])
            w_sb = const_pool.tile([IN_FEAT, N_OUT], mybir.dt.float32)
            nc.sync.dma_start(w_sb[:], w[:, :])
            b_sb = const_pool.tile([N_OUT, 1], mybir.dt.float32)
            nc.sync.dma_start(b_sb[:], b[:, :])
            w_r = const_pool.tile([IN_FEAT, N_OUT], gemm_dt)
            nc.vector.tensor_copy(w_r[:], w_sb[:])

            row0 = 0
            for s_rows in supers:
                g_tot = s_rows // 128  # row-groups in this super
                xnat = xnat_pool.tile([128, s_rows], mybir.dt.float32)
                xnat3 = xnat[:].rearrange("p (g d) -> p g d", d=IN_FEAT)
                # DRAM rows row0 + p*g_tot + g  ->  partition p, group g
                x_sup = x[row0:row0 + s_rows, :].rearrange(
                    "(p g) d -> p g d", g=g_tot)
                nc.sync.dma_start(xnat3, x_sup)

                out_sb = out_pool.tile([128, s_rows], mybir.dt.float32)
                for c0 in range(0, s_rows, PAIR):
                    cols = min(PAIR, s_rows - c0)
                    xtp = xtp_pool.tile([128, PAIR], mybir.dt.float32)
                    for i in range(cols // 128):
                        g = (c0 // 128) + i
                        nc.tensor.matmul(
                            xtp[:, i * 128:(i + 1) * 128],
                            xnat3[:, g, :],
                            ident[:],
                            is_transpose=True,
                        )
                    xts = xt_pool.tile([128, PAIR], gemm_dt)
                    nc.scalar.copy(xts[:, :cols], xtp[:, :cols])

                    outp = outp_pool.tile([128, PAIR], mybir.dt.float32)
                    for j in range(0, cols, 512):
                        nc.tensor.matmul(outp[:N_OUT, j:j + 512],
                                         w_r[:], xts[:, j:j + 512])
                    nc.vector.tensor_scalar_add(
                        out_sb[:N_OUT, c0:c0 + cols],
                        outp[:N_OUT, :cols],
                        b_sb[:, :],
                    )
                    # out-DMAs avoid the SP ring (whose data dependency
                    # would head-of-line block input DMAs); alternating the
                    # gpsimd/scalar rings doubles issue parallelism
                if row0 >= rows_pad * 3 // 4:
                    rot = [nc.gpsimd, nc.scalar, nc.sync]
                else:
                    rot = [nc.gpsimd, nc.scalar]
                out_eng = rot[row0 // 2048 % len(rot)]
                out_eng.dma_start(out[:, row0:row0 + s_rows],
                                  out_sb[:N_OUT, :s_rows])

                row0 += s_rows

    nc.compile()
    return nc


_NC_CACHE = {}


def _get_nc():
    if "nc" not in _NC_CACHE:
        _NC_CACHE["nc"] = build_nc()
    return _NC_CACHE["nc"]


def _unpermute_cols(out_t, supers):
    """Invert the grouped-row loading permutation, per super-batch."""
    parts = []
    col0 = 0
    for s_rows in supers:
        g_tot = s_rows // 128
        blk = out_t[:, col0:col0 + s_rows]
        # column g*128 + p  ->  row p*g_tot + g
        parts.append(blk.reshape(N_OUT, g_tot, 128).transpose(0, 2, 1)
                     .reshape(N_OUT, s_rows))
        col0 += s_rows
    return np.concatenate(parts, axis=1)


def kernel(x, W_cls, b_cls, W_obj, b_obj, W_reg, b_reg):
    x = np.asarray(x, dtype=np.float32)
    w_all = np.concatenate(
        [np.asarray(W_cls, np.float32), np.asarray(W_reg, np.float32),
         np.asarray(W_obj, np.float32)], axis=1)
    b_all = np.concatenate(
        [np.asarray(b_cls, np.float32), np.asarray(b_reg, np.float32),
         np.asarray(b_obj, np.float32)]).reshape(N_OUT, 1)

    x_sh = x.reshape(N_CORES, ROWS_PER_CORE, IN_FEAT)
    x_pad = np.zeros((N_CORES, ROWS_PAD, IN_FEAT), dtype=np.float32)
    x_pad[:, :ROWS_PER_CORE] = x_sh

    in_maps = [{"x": x_pad[i], "w": w_all, "b": b_all} for i in range(N_CORES)]

    nc = _get_nc()
    res = run_bass_kernel_spmd(nc, in_maps, core_ids=list(range(N_CORES)))

    out_t = np.concatenate(
        [_unpermute_cols(res.results[i]["out"], SUPERS)[:, :ROWS_PER_CORE]
         for i in range(N_CORES)],
        axis=1)  # [108, 400000]
    out = out_t.T  # view
    cls_pred = np.ascontiguousarray(out[:, :60])
    box_reg = np.ascontiguousarray(out[:, 60:102])
    objness = np.ascontiguousarray(out[:, 102:108])
    return cls_pred, box_reg, objness
